# revision 3
# baseline (speedup 1.0000x reference)
"""DeepClusteringLoss Trainium2 kernel.

loss = (||V^T V||_F^2 - 2 ||V^T E||_F^2 + ||E^T E||_F^2) / (B*N)
summed over batch, with E = embeddings.reshape(B, N, D), V =
assignments.reshape(B, N, S), N = F*T.

Sharding: data-parallel over batch; each of the 8 cores handles one batch
element.  On-core, the combined matrix W = [V | E] (N x 44) is streamed
through the PE array in 1024 chunks of 128 rows, accumulating the full
Gram G = W^T W (44 x 44) in PSUM.  The per-core scalar partial
loss = ||G||^2 - 4 ||B||^2 (B = V^T E block) is reduced on-device; the
host sums the 8 partials (the "all-reduce") and divides by B*N.
"""

import os
from contextlib import ExitStack

import numpy as np

import concourse.bacc as bacc
import concourse.mybir as mybir
import concourse.tile as tile
from concourse.bass_utils import run_bass_kernel_spmd

B, F, T, D, S = 8, 256, 512, 40, 4
N = F * T              # rows per core (131072)
SD = S + D             # 44 combined features
P = 128                # partitions / chunk rows
U = 64                 # chunks per block
BLOCK = P * U          # rows per block (8192)
NB = N // BLOCK        # blocks per core (16)
N_CORES = 8

# matmul dtype knob: float32 (exact, PE 4 cyc/row) or float16 (PE 1 cyc/row)
MM_DT_NAME = os.environ.get("KERNEL_MM_DT", "float32")

_nc_cache = {}


def _build_nc(mm_dt_name: str):
    mm_dt = getattr(mybir.dt, mm_dt_name)
    f32 = mybir.dt.float32
    cast = mm_dt != f32

    nc = bacc.Bacc("TRN2", target_bir_lowering=False, debug=False)
    E = nc.dram_tensor("embeddings", (N, D), f32, kind="ExternalInput")
    V = nc.dram_tensor("assignments", (N, S), f32, kind="ExternalInput")
    OUT = nc.dram_tensor("partial", (1, 1), f32, kind="ExternalOutput")

    with tile.TileContext(nc) as tc, ExitStack() as ctx:
        io_pool = ctx.enter_context(tc.tile_pool(name="io", bufs=3))
        w_pool = ctx.enter_context(tc.tile_pool(name="w", bufs=3))
        psum_pool = ctx.enter_context(tc.tile_pool(name="ps", bufs=1, space="PSUM"))
        g_ps = psum_pool.tile([SD, SD], f32, tag="g")

        for blk in range(NB):
            r0 = blk * BLOCK
            e_ap = E[r0:r0 + BLOCK, :].rearrange("(p u) d -> p (u d)", p=P)
            v_ap = V[r0:r0 + BLOCK, :].rearrange("(p u) s -> p (u s)", p=P)
            e_t = io_pool.tile([P, U * D], mm_dt, tag="e")
            v_t = io_pool.tile([P, U * S], mm_dt, tag="v")
            if cast:
                nc.gpsimd.dma_start(out=e_t[:], in_=e_ap)
                nc.gpsimd.dma_start(out=v_t[:], in_=v_ap)
            else:
                nc.sync.dma_start(out=e_t[:], in_=e_ap)
                nc.sync.dma_start(out=v_t[:], in_=v_ap)

            # Interleave into per-chunk [V_u | E_u] blocks of 44 columns.
            w_t = w_pool.tile([P, U * SD], mm_dt, tag="w")
            w3 = w_t[:].rearrange("p (u c) -> p u c", c=SD)
            nc.vector.tensor_copy(
                w3[:, :, S:SD], e_t[:].rearrange("p (u d) -> p u d", d=D)
            )
            nc.scalar.copy(
                w3[:, :, 0:S], v_t[:].rearrange("p (u s) -> p u s", s=S)
            )

            for u in range(U):
                wu = w_t[:, u * SD:(u + 1) * SD]
                nc.tensor.matmul(
                    g_ps[:], wu, wu,
                    start=(blk == 0 and u == 0),
                    stop=(blk == NB - 1 and u == U - 1),
                )

        # Epilogue: partial = sum(G^2) - 4 * sum(B^2), B = G[0:S, S:SD]
        ep = ctx.enter_context(tc.tile_pool(name="ep", bufs=1))
        g_sb = ep.tile([SD, SD], f32, tag="gsb")
        nc.vector.tensor_copy(g_sb[:], g_ps[:])
        g2 = ep.tile([SD, SD], f32, tag="g2")
        nc.vector.tensor_mul(g2[:], g_sb[:], g_sb[:])
        colsum = ep.tile([SD, 1], f32, tag="cs")
        nc.vector.reduce_sum(colsum[:], g2[:], axis=mybir.AxisListType.X)
        bcol = ep.tile([S, 1], f32, tag="bc")
        nc.vector.reduce_sum(bcol[:], g2[0:S, S:SD], axis=mybir.AxisListType.X)
        nc.scalar.mul(bcol[:], bcol[:], -4.0)
        ones = ep.tile([SD, 1], f32, tag="on")
        nc.vector.memset(ones[:], 1.0)
        s_ps = psum_pool.tile([1, 1], f32, tag="s")
        nc.tensor.matmul(s_ps[:], colsum[:], ones[:], start=True, stop=False)
        nc.tensor.matmul(s_ps[:], bcol[:], ones[0:S, :], start=False, stop=True)
        res = ep.tile([1, 1], f32, tag="r")
        nc.vector.tensor_copy(res[:], s_ps[:])
        nc.sync.dma_start(out=OUT[:, :], in_=res[:])

    nc.finalize()
    return nc


def _get_nc():
    if MM_DT_NAME not in _nc_cache:
        _nc_cache[MM_DT_NAME] = _build_nc(MM_DT_NAME)
    return _nc_cache[MM_DT_NAME]


def _run(embeddings: np.ndarray, assignments: np.ndarray, trace: bool = False):
    nc = _get_nc()
    in_maps = []
    for i in range(N_CORES):
        in_maps.append({
            "embeddings": np.ascontiguousarray(
                embeddings[i].reshape(N, D).astype(np.float32, copy=False)),
            "assignments": np.ascontiguousarray(
                assignments[i].reshape(N, S).astype(np.float32, copy=False)),
        })
    res = run_bass_kernel_spmd(
        nc, in_maps, core_ids=list(range(N_CORES)), trace=trace
    )
    partials = [float(r["partial"][0, 0]) for r in res.results]
    total = np.float32(np.sum(np.asarray(partials, dtype=np.float64)) / (B * N))
    return np.asarray(total, dtype=np.float32), res


def kernel(embeddings: np.ndarray, assignments: np.ndarray) -> np.ndarray:
    out, _ = _run(embeddings, assignments, trace=False)
    return out
